# revision 11
# baseline (speedup 1.0000x reference)
"""Trainium2 Bass kernel for a Neural Additive Model (dense per-feature MLPs).

Math (per batch row b, feature f):
    h1 = relu(x[b,f] * W1[f] + b1[f])          # [128]
    h2 = relu(W2[f]^T h1 + b2[f])              # [64]
    h3 = relu(W3[f]^T h2 + b3[f])              # [32]
    y  = sum_f (W4[f]^T h3 + b4[f]) + bias     # scalar
Output: [B, 1].

Distribution: data-parallel over batch across 8 NeuronCores (B=8192 -> 1024
per core), weights replicated; no collectives, host concatenates outputs.

Per-core dataflow ([hidden-on-partition, batch-on-free] layout):
  L1: PE outer products (K=2, bias folded via an interleaved ones row),
      features pair-pipelined, alternating row tile positions.
  L1/L2/L3 PSUM evacuation: relu (+bias for L2/L3) fused into the single
      PSUM->SBUF pass, split between ScalarE (ACT) and VectorE (DVE).
  L2: K=128,M=64 matmuls, 2 features packed via column tiling.
  L3: K=64,M=32 matmuls, 4 features packed via row+column tiling.
  L4: K=128,M=1 block-diagonal matmuls accumulating all features in PSUM,
      final cross-partition reduce via a ones-vector matmul.
"""

import os
from contextlib import ExitStack

import numpy as np

import concourse.bass as bass
import concourse.tile as tile
from concourse import bacc, mybir
from concourse.bass_utils import run_bass_kernel_spmd

F32 = mybir.dt.float32
AF = mybir.ActivationFunctionType
ALU = mybir.AluOpType

N_CORES = 8
B_CORE = 1024  # batch rows per core
NT = 512  # moving-dim tile (one fp32 PSUM bank)

# matmul input dtype: float32r streams 1 col/cycle (vs 4 for float32)
MM_DT = mybir.dt.float32r


def build_program(n_pairs=128, b_core=B_CORE, evac_split=4):
    """Build the per-core Bass program (SPMD: same program on all cores).

    n_pairs: number of feature pairs (F/2).
    evac_split: ACT evacuates the second L1 tile on pairs where
        p % evac_split != evac_split-1; DVE otherwise.
    """
    assert n_pairs % 2 == 0
    n_quads = n_pairs // 2
    nt2 = 2 * NT
    assert b_core == 2 * NT

    nc = bacc.Bacc("TRN2", target_bir_lowering=False, debug=False)

    xti = nc.dram_tensor("xti", [n_pairs, 2, 2, b_core], MM_DT, kind="ExternalInput")
    w1b = nc.dram_tensor("w1b", [n_pairs, 2, 2, 128], MM_DT, kind="ExternalInput")
    w2p = nc.dram_tensor("w2p", [n_pairs, 128, 256], MM_DT, kind="ExternalInput")
    w3p = nc.dram_tensor("w3p", [n_quads, 128, 256], MM_DT, kind="ExternalInput")
    w4p = nc.dram_tensor("w4p", [128, n_quads], MM_DT, kind="ExternalInput")
    b2p = nc.dram_tensor("b2p", [128, n_pairs], F32, kind="ExternalInput")
    b3p = nc.dram_tensor("b3p", [128, n_quads], F32, kind="ExternalInput")
    b4s = nc.dram_tensor("b4s", [1, 1], F32, kind="ExternalInput")
    out = nc.dram_tensor("out", [1, b_core], F32, kind="ExternalOutput")

    with tile.TileContext(nc) as tc, ExitStack() as ctx:
        statics = ctx.enter_context(tc.tile_pool(name="statics", bufs=1))
        xpool = ctx.enter_context(tc.tile_pool(name="xpool", bufs=3))
        w1pool = ctx.enter_context(tc.tile_pool(name="w1pool", bufs=3))
        w2pool = ctx.enter_context(tc.tile_pool(name="w2pool", bufs=3))
        w3pool = ctx.enter_context(tc.tile_pool(name="w3pool", bufs=2))
        h1pool = ctx.enter_context(tc.tile_pool(name="h1pool", bufs=2))
        h2pool = ctx.enter_context(tc.tile_pool(name="h2pool", bufs=3))
        h3pool = ctx.enter_context(tc.tile_pool(name="h3pool", bufs=2))
        finpool = ctx.enter_context(tc.tile_pool(name="finpool", bufs=1))
        pswork = ctx.enter_context(tc.tile_pool(name="pswork", bufs=3, space="PSUM"))
        psacc = ctx.enter_context(tc.tile_pool(name="psacc", bufs=1, space="PSUM"))

        # static staging
        b2s = statics.tile([128, n_pairs], F32, tag="b2s")
        nc.sync.dma_start(out=b2s[:, :], in_=b2p[:, :])
        b3s = statics.tile([128, n_quads], F32, tag="b3s")
        nc.sync.dma_start(out=b3s[:, :], in_=b3p[:, :])
        w4s = statics.tile([128, n_quads], MM_DT, tag="w4s")
        nc.sync.dma_start(out=w4s[:, :], in_=w4p[:, :])
        b4sb = statics.tile([128, 1], F32, tag="b4sb")
        nc.sync.dma_start(out=b4sb[0:1, 0:1], in_=b4s[:, :])

        # L4 accumulator (persistent across the whole feature loop)
        acc = psacc.tile([128, b_core], F32, tag="acc")

        h2_prev = None
        for p in range(n_pairs):
            ro = 64 * (p % 2)  # row-position base: {0,32} or {64,96}
            q = p // 2

            # ---- stream inputs/weights for this pair ----
            xst = xpool.tile([128, b_core], MM_DT, tag="xst")
            nc.sync.dma_start(out=xst[ro : ro + 2, :], in_=xti[p, 0])
            nc.sync.dma_start(out=xst[ro + 32 : ro + 34, :], in_=xti[p, 1])
            w1st = w1pool.tile([128, 128], MM_DT, tag="w1st")
            nc.sync.dma_start(out=w1st[ro : ro + 2, :], in_=w1b[p, 0])
            nc.sync.dma_start(out=w1st[ro + 32 : ro + 34, :], in_=w1b[p, 1])
            w2st = w2pool.tile([128, 256], MM_DT, tag="w2st")
            nc.sync.dma_start(out=w2st[:, :], in_=w2p[p])
            if p % 2 == 0:
                w3st = w3pool.tile([128, 256], MM_DT, tag="w3st")
                nc.sync.dma_start(out=w3st[:, :], in_=w3p[q])

            # ---- L1: z1 = W1 (x) x + b1 (x) ones  (outer products, K=2) ----
            zl1a = pswork.tile([128, b_core], F32, tag="work")
            zl1b = pswork.tile([128, b_core], F32, tag="work")
            for nt in range(2):
                s = slice(nt * NT, (nt + 1) * NT)
                nc.tensor.matmul(
                    zl1a[:, s],
                    w1st[ro : ro + 2, :],
                    xst[ro : ro + 2, s],
                    tile_position=(ro, 0),
                )
                nc.tensor.matmul(
                    zl1b[:, s],
                    w1st[ro + 32 : ro + 34, :],
                    xst[ro + 32 : ro + 34, s],
                    tile_position=(ro + 32, 0),
                )

            # ---- L1 evacuation: h1 = relu(z1), PSUM -> SBUF ----
            h1 = h1pool.tile([128, 2 * b_core], MM_DT, tag="h1")
            nc.scalar.activation(
                out=h1[:, 0:b_core], in_=zl1a[:, :], func=AF.Relu
            )
            if p % evac_split != evac_split - 1:
                nc.scalar.activation(
                    out=h1[:, b_core : 2 * b_core], in_=zl1b[:, :], func=AF.Relu
                )
            else:
                nc.vector.tensor_scalar(
                    out=h1[:, b_core : 2 * b_core],
                    in0=zl1b[:, :],
                    scalar1=0.0,
                    scalar2=None,
                    op0=ALU.max,
                )

            # ---- L2: z2 = W2^T h1; 2 features per PSUM tile via two
            # accumulating block matmuls (fp32r forbids column tiling) ----
            zl2 = pswork.tile([128, b_core], F32, tag="work")
            for nt in range(2):
                s = slice(nt * NT, (nt + 1) * NT)
                nc.tensor.matmul(
                    zl2[:, s],
                    w2st[:, 0:128],
                    h1[:, nt * NT : (nt + 1) * NT],
                    start=True,
                    stop=False,
                )
                nc.tensor.matmul(
                    zl2[:, s],
                    w2st[:, 128:256],
                    h1[:, b_core + nt * NT : b_core + (nt + 1) * NT],
                    start=False,
                    stop=True,
                )

            # ---- L2 evacuation: h2 = relu(z2 + b2) ----
            h2 = h2pool.tile([128, b_core], MM_DT, tag="h2")
            nc.vector.tensor_scalar(
                out=h2[:, :],
                in0=zl2[:, :],
                scalar1=b2s[:, p : p + 1],
                scalar2=0.0,
                op0=ALU.add,
                op1=ALU.max,
            )

            if p % 2 == 0:
                h2_prev = h2
                continue

            # ---- L3 (per quad): K=64, M=32, 4 features row+col packed ----
            h2a, h2b = h2_prev, h2
            h3 = h3pool.tile([128, b_core], MM_DT, tag="h3")
            zl3 = pswork.tile([128, b_core], F32, tag="work")
            for nt in range(2):
                s = slice(nt * NT, (nt + 1) * NT)
                nc.tensor.matmul(
                    zl3[:, s],
                    w3st[:, 0:128],
                    h2a[:, s],
                    start=True,
                    stop=False,
                )
                nc.tensor.matmul(
                    zl3[:, s],
                    w3st[:, 128:256],
                    h2b[:, s],
                    start=False,
                    stop=True,
                )
            for nt in range(2):
                s = slice(nt * NT, (nt + 1) * NT)
                # h3 = relu(z3 + b3)
                nc.vector.tensor_scalar(
                    out=h3[:, s],
                    in0=zl3[:, s],
                    scalar1=b3s[:, q : q + 1],
                    scalar2=0.0,
                    op0=ALU.add,
                    op1=ALU.max,
                )

            # ---- L4: y += W4^T h3 (K=128, M=1), accumulate in PSUM ----
            first = q == 0
            last = q == n_quads - 1
            nc.tensor.matmul(
                acc[0:1, 0:NT],
                w4s[:, q : q + 1],
                h3[:, 0:NT],
                start=first,
                stop=last,
            )
            nc.tensor.matmul(
                acc[0:1, NT : 2 * NT],
                w4s[:, q : q + 1],
                h3[:, NT : 2 * NT],
                start=first,
                stop=last,
            )

        # ---- final: out[b] = acc[0, b] + (sum(b4) + bias) ----
        outsb = finpool.tile([128, b_core], F32, tag="outsb")
        nc.vector.tensor_scalar(
            out=outsb[0:1, :],
            in0=acc[0:1, :],
            scalar1=b4sb[0:1, 0:1],
            scalar2=None,
            op0=ALU.add,
        )
        nc.sync.dma_start(out=out[:, :], in_=outsb[0:1, :])

    nc.compile()
    return nc


def pack_shared(W1, b1, W2, b2, W3, b3, W4, b4, bias, n_pairs):
    """Host-side packing of weights into the layouts the kernel streams."""
    n_quads = n_pairs // 2
    f4 = np.float32

    w1b = np.empty((n_pairs, 2, 2, 128), f4)
    for s in range(2):
        w1b[:, s, 0, :] = W1[s::2][:n_pairs]
        w1b[:, s, 1, :] = b1[s::2][:n_pairs]

    # L2 lhsT pair: pass1 [128,128] = [W2[f_a] | 0]; pass2 = [0 | W2[f_b]]
    w2p = np.zeros((n_pairs, 128, 256), f4)
    w2p[:, :, 0:64] = W2[0 : 2 * n_pairs : 2]
    w2p[:, :, 192:256] = W2[1 : 2 * n_pairs : 2]

    # L3 lhsT quad: pass1 [128,128] = [diag(W3a,W3b) | 0];
    # pass2 [128,128] = [0 | diag(W3c,W3d)]
    w3p = np.zeros((n_quads, 128, 256), f4)
    w3p[:, 0:64, 0:32] = W3[0 : 4 * n_quads : 4]
    w3p[:, 64:128, 32:64] = W3[1 : 4 * n_quads : 4]
    w3p[:, 0:64, 192:224] = W3[2 : 4 * n_quads : 4]
    w3p[:, 64:128, 224:256] = W3[3 : 4 * n_quads : 4]

    w4f = W4[:, :, 0]  # [F, 32]
    w4p = np.empty((128, n_quads), f4)
    b3t = np.empty((128, n_quads), f4)
    for i in range(4):
        w4p[32 * i : 32 * (i + 1), :] = w4f[i : 4 * n_quads : 4].T
        b3t[32 * i : 32 * (i + 1), :] = b3[i : 4 * n_quads : 4].T

    b2t = np.empty((128, n_pairs), f4)
    b2t[0:64, :] = b2[0 : 2 * n_pairs : 2].T
    b2t[64:128, :] = b2[1 : 2 * n_pairs : 2].T

    b4v = np.array([[np.sum(b4) + float(bias[0])]], f4)
    return {
        "w1b": w1b,
        "w2p": w2p,
        "w3p": w3p,
        "w4p": w4p,
        "b2p": b2t,
        "b3p": b3t,
        "b4s": b4v,
    }


def pack_x(x_core, n_pairs):
    """Per-core x staging: [pairs, slot, {x-row, ones}, B]."""
    b = x_core.shape[0]
    xT = np.ascontiguousarray(x_core.T.astype(np.float32))  # [F, B]
    xti = np.empty((n_pairs, 2, 2, b), np.float32)
    xti[:, 0, 0, :] = xT[0 : 2 * n_pairs : 2]
    xti[:, 1, 0, :] = xT[1 : 2 * n_pairs : 2]
    xti[:, :, 1, :] = 1.0
    return xti


_PROGRAM_CACHE = {}


def _get_program(n_pairs):
    key = (n_pairs, str(MM_DT))
    if key not in _PROGRAM_CACHE:
        _PROGRAM_CACHE[key] = build_program(n_pairs=n_pairs)
    return _PROGRAM_CACHE[key]


def kernel(x, W1, b1, W2, b2, W3, b3, W4, b4, bias, _trace=False):
    x = np.asarray(x, np.float32)
    args = [np.asarray(a, np.float32) for a in (W1, b1, W2, b2, W3, b3, W4, b4, bias)]
    W1, b1, W2, b2, W3, b3, W4, b4, bias = args

    B, F = x.shape
    n_pairs = F // 2
    bc = B // N_CORES
    assert bc == B_CORE, f"expected {B_CORE} rows/core, got {bc}"

    shared = pack_shared(W1, b1, W2, b2, W3, b3, W4, b4, bias, n_pairs)
    in_maps = []
    for c in range(N_CORES):
        m = dict(shared)
        m["xti"] = pack_x(x[c * bc : (c + 1) * bc], n_pairs)
        in_maps.append(m)

    nc = _get_program(n_pairs)
    res = run_bass_kernel_spmd(
        nc, in_maps, core_ids=list(range(N_CORES)), trace=_trace
    )
    out = np.concatenate(
        [res.results[c]["out"].reshape(bc, 1) for c in range(N_CORES)], axis=0
    )
    if _trace:
        kernel.last_results = res
    return out.astype(np.float32)


# revision 13
# speedup vs baseline: 1.4028x; 1.4028x over previous
"""Trainium2 Bass kernel for a Neural Additive Model (dense per-feature MLPs).

Math (per batch row b, feature f):
    h1 = relu(x[b,f] * W1[f] + b1[f])          # [128]
    h2 = relu(W2[f]^T h1 + b2[f])              # [64]
    h3 = relu(W3[f]^T h2 + b3[f])              # [32]
    y  = sum_f (W4[f]^T h3 + b4[f]) + bias     # scalar
Output: [B, 1].

Distribution: data-parallel over batch across 8 NeuronCores (B=8192 -> 1024
per core), weights replicated; no collectives, host concatenates outputs.

Per-core dataflow ([hidden-on-partition, batch-on-free] layout), v3:
  L1: PE outer products, K=5 bf16 hi/lo split (exact to ~1e-5):
      z1 = W1h(x)xh + W1h(x)xl + W1l(x)xh + b1h(x)1 + b1l(x)1.
      Features pair-pipelined, alternating row tile positions.
  L1/L2/L3 PSUM evacuation: relu (+bias for L2/L3) fused into the single
      PSUM->SBUF pass; ACT takes h1, DVE takes h2/h3.
  L2: bf16 K=128,M=64 matmuls, 2 features packed via column tiling.
  L3: bf16 K=64,M=32 matmuls, 4 features packed via row+column tiling.
  L4: bf16 K=128,M=1 matmuls accumulating all features into one PSUM bank
      (nt0 at partition 0, nt1 at partition 32 via column position 32);
      a zero dummy matmul opens the accumulation group.
"""

from contextlib import ExitStack

import numpy as np
import ml_dtypes

import concourse.bass as bass
import concourse.tile as tile
from concourse import bacc, mybir
from concourse.bass_utils import run_bass_kernel_spmd

F32 = mybir.dt.float32
BF16 = mybir.dt.float16
AF = mybir.ActivationFunctionType
ALU = mybir.AluOpType
BF = np.float16

N_CORES = 8
B_CORE = 1024  # batch rows per core
NT = 512  # moving-dim tile (one fp32 PSUM bank)


def build_program(n_pairs=128, b_core=B_CORE):
    """Build the per-core Bass program (SPMD: same program on all cores)."""
    assert n_pairs % 2 == 0
    n_quads = n_pairs // 2

    nc = bacc.Bacc("TRN2", target_bir_lowering=False, debug=False)

    xti = nc.dram_tensor("xti", [n_pairs, 2, 5, b_core], BF16, kind="ExternalInput")
    w1b = nc.dram_tensor("w1b", [n_pairs, 2, 5, 128], BF16, kind="ExternalInput")
    w2p = nc.dram_tensor("w2p", [n_pairs, 128, 128], BF16, kind="ExternalInput")
    w3p = nc.dram_tensor("w3p", [n_quads, 128, 64], BF16, kind="ExternalInput")
    w4p = nc.dram_tensor("w4p", [128, n_quads], BF16, kind="ExternalInput")
    b2p = nc.dram_tensor("b2p", [128, n_pairs], F32, kind="ExternalInput")
    b3p = nc.dram_tensor("b3p", [128, n_quads], F32, kind="ExternalInput")
    b4s = nc.dram_tensor("b4s", [1, 1], F32, kind="ExternalInput")
    out = nc.dram_tensor("out", [1, b_core], F32, kind="ExternalOutput")

    with tile.TileContext(nc) as tc, ExitStack() as ctx:
        statics = ctx.enter_context(tc.tile_pool(name="statics", bufs=1))
        xpool = ctx.enter_context(tc.tile_pool(name="xpool", bufs=3))
        w1pool = ctx.enter_context(tc.tile_pool(name="w1pool", bufs=3))
        w2pool = ctx.enter_context(tc.tile_pool(name="w2pool", bufs=3))
        w3pool = ctx.enter_context(tc.tile_pool(name="w3pool", bufs=2))
        h1pool = ctx.enter_context(tc.tile_pool(name="h1pool", bufs=2))
        h2pool = ctx.enter_context(tc.tile_pool(name="h2pool", bufs=3))
        h3pool = ctx.enter_context(tc.tile_pool(name="h3pool", bufs=2))
        finpool = ctx.enter_context(tc.tile_pool(name="finpool", bufs=1))
        psl1 = ctx.enter_context(tc.tile_pool(name="psl1", bufs=2, space="PSUM"))
        psl2 = ctx.enter_context(tc.tile_pool(name="psl2", bufs=1, space="PSUM"))
        psl3 = ctx.enter_context(tc.tile_pool(name="psl3", bufs=1, space="PSUM"))
        psacc = ctx.enter_context(tc.tile_pool(name="psacc", bufs=1, space="PSUM"))

        # static staging
        b2s = statics.tile([128, n_pairs], F32, tag="b2s")
        nc.sync.dma_start(out=b2s[:, :], in_=b2p[:, :])
        b3s = statics.tile([128, n_quads], F32, tag="b3s")
        nc.sync.dma_start(out=b3s[:, :], in_=b3p[:, :])
        w4s = statics.tile([128, n_quads], BF16, tag="w4s")
        nc.sync.dma_start(out=w4s[:, :], in_=w4p[:, :])
        b4sb = statics.tile([128, 1], F32, tag="b4sb")
        nc.sync.dma_start(out=b4sb[0:1, 0:1], in_=b4s[:, :])
        zconst = statics.tile([128, NT], BF16, tag="zconst")
        nc.vector.memset(zconst[0:1, :], 0.0)

        # L4 accumulator: one bank; nt0 sums at partition 0, nt1 at 32.
        acc = psacc.tile([128, NT], F32, tag="acc")
        # dummy matmul opens the accumulation group: start=True clears
        # has_written for the bank and writes zeros to partitions 0..32, so
        # every real L4 matmul can run start=False (overwrite-then-accum).
        nc.tensor.matmul(
            acc[0:33, :], zconst[0:1, 0:33], zconst[0:1, :],
            start=True, stop=False, skip_group_check=True,
        )

        h2_prev = None
        for p in range(n_pairs):
            ro = 64 * (p % 2)  # row-position base: {0,32} or {64,96}
            q = p // 2

            # ---- stream inputs/weights for this pair ----
            xst = xpool.tile([128, b_core], BF16, tag="xst")
            nc.sync.dma_start(out=xst[ro : ro + 5, :], in_=xti[p, 0])
            nc.sync.dma_start(out=xst[ro + 32 : ro + 37, :], in_=xti[p, 1])
            w1st = w1pool.tile([128, 128], BF16, tag="w1st")
            nc.sync.dma_start(out=w1st[ro : ro + 5, :], in_=w1b[p, 0])
            nc.sync.dma_start(out=w1st[ro + 32 : ro + 37, :], in_=w1b[p, 1])
            w2st = w2pool.tile([128, 128], BF16, tag="w2st")
            nc.sync.dma_start(out=w2st[:, :], in_=w2p[p])
            if p % 2 == 0:
                w3st = w3pool.tile([128, 64], BF16, tag="w3st")
                nc.sync.dma_start(out=w3st[:, :], in_=w3p[q])

            # ---- L1: z1 via K=5 split outer products ----
            zl1a = psl1.tile([128, b_core], F32, tag="zl1")
            zl1b = psl1.tile([128, b_core], F32, tag="zl1")
            for nt in range(2):
                s = slice(nt * NT, (nt + 1) * NT)
                nc.tensor.matmul(
                    zl1a[:, s],
                    w1st[ro : ro + 5, :],
                    xst[ro : ro + 5, s],
                    tile_position=(ro, 0),
                )
                nc.tensor.matmul(
                    zl1b[:, s],
                    w1st[ro + 32 : ro + 37, :],
                    xst[ro + 32 : ro + 37, s],
                    tile_position=(ro + 32, 0),
                )

            # ---- L1 evacuation (ACT): h1 = relu(z1), PSUM -> SBUF bf16 ----
            h1 = h1pool.tile([128, 2 * b_core], BF16, tag="h1")
            nc.scalar.activation(out=h1[:, 0:b_core], in_=zl1a[:, :], func=AF.Relu)
            nc.scalar.activation(
                out=h1[:, b_core : 2 * b_core], in_=zl1b[:, :], func=AF.Relu
            )

            # ---- L2: z2 = W2^T h1 (K=128, M=64, two features col-packed) ----
            zl2 = psl2.tile([128, b_core], F32, tag="zl2")
            for nt in range(2):
                s = slice(nt * NT, (nt + 1) * NT)
                nc.tensor.matmul(
                    zl2[0:64, s],
                    w2st[:, 0:64],
                    h1[:, nt * NT : (nt + 1) * NT],
                    tile_position=(0, 0),
                )
                nc.tensor.matmul(
                    zl2[64:128, s],
                    w2st[:, 64:128],
                    h1[:, b_core + nt * NT : b_core + (nt + 1) * NT],
                    tile_position=(0, 64),
                )

            # ---- L2 evacuation (DVE): h2 = relu(z2 + b2) ----
            h2 = h2pool.tile([128, b_core], BF16, tag="h2")
            nc.vector.tensor_scalar(
                out=h2[:, :],
                in0=zl2[:, :],
                scalar1=b2s[:, p : p + 1],
                scalar2=0.0,
                op0=ALU.add,
                op1=ALU.max,
            )

            if p % 2 == 0:
                h2_prev = h2
                continue

            # ---- L3 (per quad): K=64, M=32, 4 features row+col packed ----
            h2a, h2b = h2_prev, h2
            h3 = h3pool.tile([128, b_core], BF16, tag="h3")
            for nt in range(2):
                s = slice(nt * NT, (nt + 1) * NT)
                zl3 = psl3.tile([128, NT], F32, tag="zl3")
                nc.tensor.matmul(
                    zl3[0:32, :], w3st[0:64, 0:32], h2a[0:64, s],
                    tile_position=(0, 0),
                )
                nc.tensor.matmul(
                    zl3[32:64, :], w3st[64:128, 0:32], h2a[64:128, s],
                    tile_position=(64, 32),
                )
                nc.tensor.matmul(
                    zl3[64:96, :], w3st[0:64, 32:64], h2b[0:64, s],
                    tile_position=(0, 64),
                )
                nc.tensor.matmul(
                    zl3[96:128, :], w3st[64:128, 32:64], h2b[64:128, s],
                    tile_position=(64, 96),
                )
                # ---- L3 evacuation (DVE): h3 = relu(z3 + b3) ----
                nc.vector.tensor_scalar(
                    out=h3[:, s],
                    in0=zl3[:, :],
                    scalar1=b3s[:, q : q + 1],
                    scalar2=0.0,
                    op0=ALU.add,
                    op1=ALU.max,
                )

            # ---- L4: y += W4^T h3 (K=128, M=1); nt0 -> partition 0,
            # nt1 -> partition 32 (column position 32), same bank ----
            nc.tensor.matmul(
                acc[0:1, :],
                w4s[:, q : q + 1],
                h3[:, 0:NT],
                tile_position=(0, 0),
                start=False,
                stop=False,
                skip_group_check=True,
            )
            nc.tensor.matmul(
                acc[32:33, :],
                w4s[:, q : q + 1],
                h3[:, NT : 2 * NT],
                tile_position=(0, 32),
                start=False,
                stop=(q == n_quads - 1),
                skip_group_check=True,
            )

        # ---- final: out[b] = acc + (sum(b4) + bias) ----
        outsb = finpool.tile([128, b_core], F32, tag="outsb")
        nc.vector.tensor_scalar(
            out=outsb[0:1, 0:NT],
            in0=acc[0:1, :],
            scalar1=b4sb[0:1, 0:1],
            scalar2=None,
            op0=ALU.add,
        )
        nc.vector.tensor_scalar(
            out=outsb[32:33, NT : 2 * NT],
            in0=acc[32:33, :],
            scalar1=b4sb[0:1, 0:1],
            scalar2=None,
            op0=ALU.add,
        )
        nc.sync.dma_start(out=out[0:1, 0:NT], in_=outsb[0:1, 0:NT])
        nc.sync.dma_start(out=out[0:1, NT : 2 * NT], in_=outsb[32:33, NT : 2 * NT])

    nc.compile()
    return nc


def _split_hi_lo(a):
    hi = a.astype(BF)
    lo = (a - hi.astype(np.float32)).astype(BF)
    return hi, lo


def pack_shared(W1, b1, W2, b2, W3, b3, W4, b4, bias, n_pairs):
    """Host-side packing of weights into the layouts the kernel streams."""
    n_quads = n_pairs // 2
    f4 = np.float32

    # L1 lhsT rows: [W1h; W1h; W1l; b1h; b1l] per feature
    w1h, w1l = _split_hi_lo(W1)
    b1h, b1l = _split_hi_lo(b1)
    w1b = np.empty((n_pairs, 2, 5, 128), BF)
    for s in range(2):
        w1b[:, s, 0, :] = w1h[s::2][:n_pairs]
        w1b[:, s, 1, :] = w1h[s::2][:n_pairs]
        w1b[:, s, 2, :] = w1l[s::2][:n_pairs]
        w1b[:, s, 3, :] = b1h[s::2][:n_pairs]
        w1b[:, s, 4, :] = b1l[s::2][:n_pairs]

    w2p = np.empty((n_pairs, 128, 128), BF)
    w2p[:, :, 0:64] = W2[0 : 2 * n_pairs : 2]
    w2p[:, :, 64:128] = W2[1 : 2 * n_pairs : 2]

    w3p = np.zeros((n_quads, 128, 64), BF)
    w3p[:, 0:64, 0:32] = W3[0 : 4 * n_quads : 4]
    w3p[:, 64:128, 0:32] = W3[1 : 4 * n_quads : 4]
    w3p[:, 0:64, 32:64] = W3[2 : 4 * n_quads : 4]
    w3p[:, 64:128, 32:64] = W3[3 : 4 * n_quads : 4]

    w4f = W4[:, :, 0]  # [F, 32]
    w4p = np.empty((128, n_quads), BF)
    b3t = np.empty((128, n_quads), f4)
    for i in range(4):
        w4p[32 * i : 32 * (i + 1), :] = w4f[i : 4 * n_quads : 4].T
        b3t[32 * i : 32 * (i + 1), :] = b3[i : 4 * n_quads : 4].T

    b2t = np.empty((128, n_pairs), f4)
    b2t[0:64, :] = b2[0 : 2 * n_pairs : 2].T
    b2t[64:128, :] = b2[1 : 2 * n_pairs : 2].T

    b4v = np.array([[np.sum(b4) + float(bias[0])]], f4)
    return {
        "w1b": w1b,
        "w2p": w2p,
        "w3p": w3p,
        "w4p": w4p,
        "b2p": b2t,
        "b3p": b3t,
        "b4s": b4v,
    }


def pack_x(x_core, n_pairs):
    """Per-core x staging rows: [xh; xl; xh; 1; 1] per feature slot."""
    b = x_core.shape[0]
    xT = np.ascontiguousarray(x_core.T.astype(np.float32))  # [F, B]
    xh, xl = _split_hi_lo(xT)
    xti = np.empty((n_pairs, 2, 5, b), BF)
    for s in range(2):
        xti[:, s, 0, :] = xh[s::2][:n_pairs]
        xti[:, s, 1, :] = xl[s::2][:n_pairs]
        xti[:, s, 2, :] = xh[s::2][:n_pairs]
    xti[:, :, 3:5, :] = BF(1.0)
    return xti


_PROGRAM_CACHE = {}


def _get_program(n_pairs):
    if n_pairs not in _PROGRAM_CACHE:
        _PROGRAM_CACHE[n_pairs] = build_program(n_pairs=n_pairs)
    return _PROGRAM_CACHE[n_pairs]


def kernel(x, W1, b1, W2, b2, W3, b3, W4, b4, bias, _trace=False):
    x = np.asarray(x, np.float32)
    args = [np.asarray(a, np.float32) for a in (W1, b1, W2, b2, W3, b3, W4, b4, bias)]
    W1, b1, W2, b2, W3, b3, W4, b4, bias = args

    B, F = x.shape
    n_pairs = F // 2
    bc = B // N_CORES
    assert bc == B_CORE, f"expected {B_CORE} rows/core, got {bc}"

    shared = pack_shared(W1, b1, W2, b2, W3, b3, W4, b4, bias, n_pairs)
    in_maps = []
    for c in range(N_CORES):
        m = dict(shared)
        m["xti"] = pack_x(x[c * bc : (c + 1) * bc], n_pairs)
        in_maps.append(m)

    nc = _get_program(n_pairs)
    res = run_bass_kernel_spmd(
        nc, in_maps, core_ids=list(range(N_CORES)), trace=_trace
    )
    out = np.concatenate(
        [res.results[c]["out"].reshape(bc, 1) for c in range(N_CORES)], axis=0
    )
    if _trace:
        kernel.last_results = res
    return out.astype(np.float32)


# revision 14
# speedup vs baseline: 1.6093x; 1.1472x over previous
"""Trainium2 Bass kernel for a Neural Additive Model (dense per-feature MLPs).

Math (per batch row b, feature f):
    h1 = relu(x[b,f] * W1[f] + b1[f])          # [128]
    h2 = relu(W2[f]^T h1 + b2[f])              # [64]
    h3 = relu(W3[f]^T h2 + b3[f])              # [32]
    y  = sum_f (W4[f]^T h3 + b4[f]) + bias     # scalar
Output: [B, 1].

Distribution: data-parallel over batch across 8 NeuronCores (B=8192 -> 1024
per core), weights replicated; no collectives, host concatenates outputs.

Per-core dataflow ([hidden-on-partition, batch-on-free] layout), v3:
  L1: PE outer products, K=5 bf16 hi/lo split (exact to ~1e-5):
      z1 = W1h(x)xh + W1h(x)xl + W1l(x)xh + b1h(x)1 + b1l(x)1.
      Features pair-pipelined, alternating row tile positions.
  L1/L2/L3 PSUM evacuation: relu (+bias for L2/L3) fused into the single
      PSUM->SBUF pass; ACT takes h1, DVE takes h2/h3.
  L2: bf16 K=128,M=64 matmuls, 2 features packed via column tiling.
  L3: bf16 K=64,M=32 matmuls, 4 features packed via row+column tiling.
  L4: bf16 K=128,M=1 matmuls accumulating all features into one PSUM bank
      (nt0 at partition 0, nt1 at partition 32 via column position 32);
      a zero dummy matmul opens the accumulation group.
"""

from contextlib import ExitStack

import numpy as np
import ml_dtypes

import concourse.bass as bass
import concourse.tile as tile
from concourse import bacc, mybir
from concourse.bass_utils import run_bass_kernel_spmd

F32 = mybir.dt.float32
BF16 = mybir.dt.float16
AF = mybir.ActivationFunctionType
ALU = mybir.AluOpType
BF = np.float16

N_CORES = 8
B_CORE = 1024  # batch rows per core
NT = 512  # moving-dim tile (one fp32 PSUM bank)


def build_program(n_pairs=128, b_core=B_CORE):
    """Build the per-core Bass program (SPMD: same program on all cores)."""
    assert n_pairs % 2 == 0
    n_quads = n_pairs // 2

    nc = bacc.Bacc("TRN2", target_bir_lowering=False, debug=False)

    xti = nc.dram_tensor("xti", [n_pairs, 2, 5, b_core + 128], BF16, kind="ExternalInput")
    w2p = nc.dram_tensor("w2p", [n_pairs, 128, 128], BF16, kind="ExternalInput")
    w3p = nc.dram_tensor("w3p", [n_quads, 128, 128], BF16, kind="ExternalInput")
    w4p = nc.dram_tensor("w4p", [128, n_quads], BF16, kind="ExternalInput")
    b2p = nc.dram_tensor("b2p", [128, n_pairs], F32, kind="ExternalInput")
    b3p = nc.dram_tensor("b3p", [128, n_quads], F32, kind="ExternalInput")
    b4s = nc.dram_tensor("b4s", [1, 1], F32, kind="ExternalInput")
    out = nc.dram_tensor("out", [1, b_core], F32, kind="ExternalOutput")

    with tile.TileContext(nc) as tc, ExitStack() as ctx:
        statics = ctx.enter_context(tc.tile_pool(name="statics", bufs=1))
        xpool = ctx.enter_context(tc.tile_pool(name="xpool", bufs=3))
        w2pool = ctx.enter_context(tc.tile_pool(name="w2pool", bufs=3))
        w3pool = ctx.enter_context(tc.tile_pool(name="w3pool", bufs=2))
        h1pool = ctx.enter_context(tc.tile_pool(name="h1pool", bufs=3))
        h2pool = ctx.enter_context(tc.tile_pool(name="h2pool", bufs=4))
        h3pool = ctx.enter_context(tc.tile_pool(name="h3pool", bufs=2))
        finpool = ctx.enter_context(tc.tile_pool(name="finpool", bufs=1))
        psl1 = ctx.enter_context(tc.tile_pool(name="psl1", bufs=2, space="PSUM"))
        psl2 = ctx.enter_context(tc.tile_pool(name="psl2", bufs=1, space="PSUM"))
        psl3 = ctx.enter_context(tc.tile_pool(name="psl3", bufs=1, space="PSUM"))
        psacc = ctx.enter_context(tc.tile_pool(name="psacc", bufs=1, space="PSUM"))

        # static staging
        b2s = statics.tile([128, n_pairs], F32, tag="b2s")
        nc.sync.dma_start(out=b2s[:, :], in_=b2p[:, :])
        b3s = statics.tile([128, n_quads], F32, tag="b3s")
        nc.sync.dma_start(out=b3s[:, :], in_=b3p[:, :])
        w4s = statics.tile([128, n_quads], BF16, tag="w4s")
        nc.sync.dma_start(out=w4s[:, :], in_=w4p[:, :])
        b4sb = statics.tile([128, 1], F32, tag="b4sb")
        nc.sync.dma_start(out=b4sb[0:1, 0:1], in_=b4s[:, :])
        zconst = statics.tile([128, NT], BF16, tag="zconst")
        nc.vector.memset(zconst[0:1, :], 0.0)

        # L4 accumulator: one bank; nt0 sums at partition 0, nt1 at 32.
        acc = psacc.tile([128, NT], F32, tag="acc")
        # dummy matmul opens the accumulation group: start=True clears
        # has_written for the bank and writes zeros to partitions 0..32, so
        # every real L4 matmul can run start=False (overwrite-then-accum).
        nc.tensor.matmul(
            acc[0:33, :], zconst[0:1, 0:33], zconst[0:1, :],
            start=True, stop=False, skip_group_check=True,
        )

        h2_prev = None
        for p in range(n_pairs):
            ro = 64 * (p % 2)  # row-position base: {0,32} or {64,96}
            q = p // 2

            # ---- stream inputs/weights for this pair ----
            # xst rows carry [x-rows | W1 columns] for the K=5 split matmul
            xst = xpool.tile([128, b_core + 128], BF16, tag="xst")
            nc.sync.dma_start(out=xst[ro : ro + 5, :], in_=xti[p, 0])
            nc.sync.dma_start(out=xst[ro + 32 : ro + 37, :], in_=xti[p, 1])
            w2st = w2pool.tile([128, 128], BF16, tag="w2st")
            nc.gpsimd.dma_start(out=w2st[:, :], in_=w2p[p])
            if p % 2 == 0:
                w3st = w3pool.tile([128, 128], BF16, tag="w3st")
                nc.gpsimd.dma_start(out=w3st[:, :], in_=w3p[q])

            # ---- L1: z1 via K=5 split outer products ----
            zl1a = psl1.tile([128, b_core], F32, tag="zl1")
            zl1b = psl1.tile([128, b_core], F32, tag="zl1")
            for nt in range(2):
                s = slice(nt * NT, (nt + 1) * NT)
                nc.tensor.matmul(
                    zl1a[:, s],
                    xst[ro : ro + 5, b_core : b_core + 128],
                    xst[ro : ro + 5, s],
                    tile_position=(ro, 0),
                )
                nc.tensor.matmul(
                    zl1b[:, s],
                    xst[ro + 32 : ro + 37, b_core : b_core + 128],
                    xst[ro + 32 : ro + 37, s],
                    tile_position=(ro + 32, 0),
                )

            # ---- L1 evacuation (ACT): h1 = relu(z1), PSUM -> SBUF bf16 ----
            h1 = h1pool.tile([128, 2 * b_core], BF16, tag="h1")
            nc.scalar.activation(out=h1[:, 0:b_core], in_=zl1a[:, :], func=AF.Relu)
            if p % 4 == 3:
                nc.vector.tensor_scalar(
                    out=h1[:, b_core : 2 * b_core],
                    in0=zl1b[:, :],
                    scalar1=0.0,
                    scalar2=None,
                    op0=ALU.max,
                )
            else:
                nc.scalar.activation(
                    out=h1[:, b_core : 2 * b_core], in_=zl1b[:, :], func=AF.Relu
                )

            # ---- L2: z2 = W2^T h1 (K=128, M=64, two features col-packed) ----
            zl2 = psl2.tile([128, b_core], F32, tag="zl2")
            for nt in range(2):
                s = slice(nt * NT, (nt + 1) * NT)
                nc.tensor.matmul(
                    zl2[0:64, s],
                    w2st[:, 0:64],
                    h1[:, nt * NT : (nt + 1) * NT],
                    tile_position=(0, 0),
                )
                nc.tensor.matmul(
                    zl2[64:128, s],
                    w2st[:, 64:128],
                    h1[:, b_core + nt * NT : b_core + (nt + 1) * NT],
                    tile_position=(0, 64),
                )

            # ---- L2 evacuation (DVE): h2 = relu(z2 + b2) ----
            h2 = h2pool.tile([128, b_core], BF16, tag="h2")
            nc.vector.tensor_scalar(
                out=h2[:, :],
                in0=zl2[:, :],
                scalar1=b2s[:, p : p + 1],
                scalar2=0.0,
                op0=ALU.add,
                op1=ALU.max,
            )

            if p % 2 == 0:
                h2_prev = h2
                continue

            # ---- L3 (per quad): K=64, M=32, 4 features row+col packed ----
            h2a, h2b = h2_prev, h2
            h3 = h3pool.tile([128, b_core], BF16, tag="h3")
            for nt in range(2):
                s = slice(nt * NT, (nt + 1) * NT)
                zl3 = psl3.tile([128, NT], F32, tag="zl3")
                nc.tensor.matmul(
                    zl3[0:64, :], w3st[:, 0:64], h2a[:, s],
                    tile_position=(0, 0),
                )
                nc.tensor.matmul(
                    zl3[64:128, :], w3st[:, 64:128], h2b[:, s],
                    tile_position=(0, 64),
                )
                # ---- L3 evacuation (DVE): h3 = relu(z3 + b3) ----
                nc.vector.tensor_scalar(
                    out=h3[:, s],
                    in0=zl3[:, :],
                    scalar1=b3s[:, q : q + 1],
                    scalar2=0.0,
                    op0=ALU.add,
                    op1=ALU.max,
                )

            # ---- L4: y += W4^T h3 (K=128, M=1); nt0 -> partition 0,
            # nt1 -> partition 32 (column position 32), same bank ----
            nc.tensor.matmul(
                acc[0:1, :],
                w4s[:, q : q + 1],
                h3[:, 0:NT],
                tile_position=(0, 0),
                start=False,
                stop=False,
                skip_group_check=True,
            )
            nc.tensor.matmul(
                acc[32:33, :],
                w4s[:, q : q + 1],
                h3[:, NT : 2 * NT],
                tile_position=(0, 32),
                start=False,
                stop=(q == n_quads - 1),
                skip_group_check=True,
            )

        # ---- final: out[b] = acc + (sum(b4) + bias) ----
        outsb = finpool.tile([128, b_core], F32, tag="outsb")
        nc.vector.tensor_scalar(
            out=outsb[0:1, 0:NT],
            in0=acc[0:1, :],
            scalar1=b4sb[0:1, 0:1],
            scalar2=None,
            op0=ALU.add,
        )
        nc.vector.tensor_scalar(
            out=outsb[32:33, NT : 2 * NT],
            in0=acc[32:33, :],
            scalar1=b4sb[0:1, 0:1],
            scalar2=None,
            op0=ALU.add,
        )
        nc.sync.dma_start(out=out[0:1, 0:NT], in_=outsb[0:1, 0:NT])
        nc.sync.dma_start(out=out[0:1, NT : 2 * NT], in_=outsb[32:33, NT : 2 * NT])

    nc.compile()
    return nc


def _split_hi_lo(a):
    hi = a.astype(BF)
    lo = (a - hi.astype(np.float32)).astype(BF)
    return hi, lo


def pack_shared(W1, b1, W2, b2, W3, b3, W4, b4, bias, n_pairs):
    """Host-side packing of weights into the layouts the kernel streams."""
    n_quads = n_pairs // 2
    f4 = np.float32

    # L1 lhsT rows: [W1h; W1h; W1l; b1h; b1l] per feature
    w1h, w1l = _split_hi_lo(W1)
    b1h, b1l = _split_hi_lo(b1)
    w1b = np.empty((n_pairs, 2, 5, 128), BF)
    for s in range(2):
        w1b[:, s, 0, :] = w1h[s::2][:n_pairs]
        w1b[:, s, 1, :] = w1h[s::2][:n_pairs]
        w1b[:, s, 2, :] = w1l[s::2][:n_pairs]
        w1b[:, s, 3, :] = b1h[s::2][:n_pairs]
        w1b[:, s, 4, :] = b1l[s::2][:n_pairs]

    w2p = np.empty((n_pairs, 128, 128), BF)
    w2p[:, :, 0:64] = W2[0 : 2 * n_pairs : 2]
    w2p[:, :, 64:128] = W2[1 : 2 * n_pairs : 2]

    # block-diag over the h2 pair tiles: cols 0:63 <- (W3a, W3b),
    # cols 64:127 <- (W3c, W3d)
    w3p = np.zeros((n_quads, 128, 128), BF)
    w3p[:, 0:64, 0:32] = W3[0 : 4 * n_quads : 4]
    w3p[:, 64:128, 32:64] = W3[1 : 4 * n_quads : 4]
    w3p[:, 0:64, 64:96] = W3[2 : 4 * n_quads : 4]
    w3p[:, 64:128, 96:128] = W3[3 : 4 * n_quads : 4]

    w4f = W4[:, :, 0]  # [F, 32]
    w4p = np.empty((128, n_quads), BF)
    b3t = np.empty((128, n_quads), f4)
    for i in range(4):
        w4p[32 * i : 32 * (i + 1), :] = w4f[i : 4 * n_quads : 4].T
        b3t[32 * i : 32 * (i + 1), :] = b3[i : 4 * n_quads : 4].T

    b2t = np.empty((128, n_pairs), f4)
    b2t[0:64, :] = b2[0 : 2 * n_pairs : 2].T
    b2t[64:128, :] = b2[1 : 2 * n_pairs : 2].T

    b4v = np.array([[np.sum(b4) + float(bias[0])]], f4)
    return {
        "_w1b": w1b,
        "w2p": w2p,
        "w3p": w3p,
        "w4p": w4p,
        "b2p": b2t,
        "b3p": b3t,
        "b4s": b4v,
    }


def pack_x(x_core, n_pairs, w1b):
    """Per-core x staging rows: [xh; xl; xh; 1; 1 | W1/b1 cols] per slot."""
    b = x_core.shape[0]
    xT = np.ascontiguousarray(x_core.T.astype(np.float32))  # [F, B]
    xh, xl = _split_hi_lo(xT)
    xti = np.empty((n_pairs, 2, 5, b + 128), BF)
    for s in range(2):
        xti[:, s, 0, 0:b] = xh[s::2][:n_pairs]
        xti[:, s, 1, 0:b] = xl[s::2][:n_pairs]
        xti[:, s, 2, 0:b] = xh[s::2][:n_pairs]
    xti[:, :, 3:5, 0:b] = BF(1.0)
    xti[:, :, :, b:] = w1b
    return xti


_PROGRAM_CACHE = {}


def _get_program(n_pairs):
    if n_pairs not in _PROGRAM_CACHE:
        _PROGRAM_CACHE[n_pairs] = build_program(n_pairs=n_pairs)
    return _PROGRAM_CACHE[n_pairs]


def kernel(x, W1, b1, W2, b2, W3, b3, W4, b4, bias, _trace=False):
    x = np.asarray(x, np.float32)
    args = [np.asarray(a, np.float32) for a in (W1, b1, W2, b2, W3, b3, W4, b4, bias)]
    W1, b1, W2, b2, W3, b3, W4, b4, bias = args

    B, F = x.shape
    n_pairs = F // 2
    bc = B // N_CORES
    assert bc == B_CORE, f"expected {B_CORE} rows/core, got {bc}"

    shared = pack_shared(W1, b1, W2, b2, W3, b3, W4, b4, bias, n_pairs)
    w1b = shared.pop("_w1b")
    in_maps = []
    for c in range(N_CORES):
        m = dict(shared)
        m["xti"] = pack_x(x[c * bc : (c + 1) * bc], n_pairs, w1b)
        in_maps.append(m)

    nc = _get_program(n_pairs)
    res = run_bass_kernel_spmd(
        nc, in_maps, core_ids=list(range(N_CORES)), trace=_trace
    )
    out = np.concatenate(
        [res.results[c]["out"].reshape(bc, 1) for c in range(N_CORES)], axis=0
    )
    if _trace:
        kernel.last_results = res
    return out.astype(np.float32)


# revision 17
# speedup vs baseline: 1.6102x; 1.0006x over previous
"""Trainium2 Bass kernel for a Neural Additive Model (dense per-feature MLPs).

Math (per batch row b, feature f):
    h1 = relu(x[b,f] * W1[f] + b1[f])          # [128]
    h2 = relu(W2[f]^T h1 + b2[f])              # [64]
    h3 = relu(W3[f]^T h2 + b3[f])              # [32]
    y  = sum_f (W4[f]^T h3 + b4[f]) + bias     # scalar
Output: [B, 1].

Distribution: data-parallel over batch across 8 NeuronCores (B=8192 -> 1024
per core), weights replicated; no collectives, host concatenates outputs.

Per-core dataflow ([hidden-on-partition, batch-on-free] layout), v3:
  L1: PE outer products, K=5 bf16 hi/lo split (exact to ~1e-5):
      z1 = W1h(x)xh + W1h(x)xl + W1l(x)xh + b1h(x)1 + b1l(x)1.
      Features pair-pipelined, alternating row tile positions.
  L1/L2/L3 PSUM evacuation: relu (+bias for L2/L3) fused into the single
      PSUM->SBUF pass; ACT takes h1, DVE takes h2/h3.
  L2: bf16 K=128,M=64 matmuls, 2 features packed via column tiling.
  L3: bf16 K=64,M=32 matmuls, 4 features packed via row+column tiling.
  L4: bf16 K=128,M=1 matmuls accumulating all features into one PSUM bank
      (nt0 at partition 0, nt1 at partition 32 via column position 32);
      a zero dummy matmul opens the accumulation group.
"""

from contextlib import ExitStack

import numpy as np
import ml_dtypes

import concourse.bass as bass
import concourse.tile as tile
from concourse import bacc, mybir
from concourse.bass_utils import run_bass_kernel_spmd

F32 = mybir.dt.float32
BF16 = mybir.dt.float16
AF = mybir.ActivationFunctionType
ALU = mybir.AluOpType
BF = np.float16

N_CORES = 8
B_CORE = 1024  # batch rows per core
NT = 512  # moving-dim tile (one fp32 PSUM bank)


def build_program(n_pairs=128, b_core=B_CORE):
    """Build the per-core Bass program (SPMD: same program on all cores)."""
    assert n_pairs % 2 == 0
    n_quads = n_pairs // 2

    nc = bacc.Bacc("TRN2", target_bir_lowering=False, debug=False)

    xti = nc.dram_tensor("xti", [n_pairs, 2, 5, b_core + 128], BF16, kind="ExternalInput")
    w2p = nc.dram_tensor("w2p", [n_pairs, 128, 128], BF16, kind="ExternalInput")
    w3p = nc.dram_tensor("w3p", [n_quads, 128, 128], BF16, kind="ExternalInput")
    w4p = nc.dram_tensor("w4p", [128, n_quads], BF16, kind="ExternalInput")
    b2p = nc.dram_tensor("b2p", [128, n_pairs], F32, kind="ExternalInput")
    b3p = nc.dram_tensor("b3p", [128, n_quads], F32, kind="ExternalInput")
    b4s = nc.dram_tensor("b4s", [1, 1], F32, kind="ExternalInput")
    out = nc.dram_tensor("out", [1, b_core], F32, kind="ExternalOutput")

    with tile.TileContext(nc) as tc, ExitStack() as ctx:
        statics = ctx.enter_context(tc.tile_pool(name="statics", bufs=1))
        xpool = ctx.enter_context(tc.tile_pool(name="xpool", bufs=3))
        w2pool = ctx.enter_context(tc.tile_pool(name="w2pool", bufs=3))
        w3pool = ctx.enter_context(tc.tile_pool(name="w3pool", bufs=2))
        h1pool = ctx.enter_context(tc.tile_pool(name="h1pool", bufs=3))
        h2pool = ctx.enter_context(tc.tile_pool(name="h2pool", bufs=4))
        h3pool = ctx.enter_context(tc.tile_pool(name="h3pool", bufs=2))
        finpool = ctx.enter_context(tc.tile_pool(name="finpool", bufs=1))
        psl1 = ctx.enter_context(tc.tile_pool(name="psl1", bufs=2, space="PSUM"))
        psl2 = ctx.enter_context(tc.tile_pool(name="psl2", bufs=1, space="PSUM"))
        psl3 = ctx.enter_context(tc.tile_pool(name="psl3", bufs=1, space="PSUM"))
        psacc = ctx.enter_context(tc.tile_pool(name="psacc", bufs=1, space="PSUM"))

        # static staging
        b2s = statics.tile([128, n_pairs], F32, tag="b2s")
        nc.sync.dma_start(out=b2s[:, :], in_=b2p[:, :])
        b3s = statics.tile([128, n_quads], F32, tag="b3s")
        nc.sync.dma_start(out=b3s[:, :], in_=b3p[:, :])
        w4s = statics.tile([128, n_quads], BF16, tag="w4s")
        nc.sync.dma_start(out=w4s[:, :], in_=w4p[:, :])
        b4sb = statics.tile([128, 1], F32, tag="b4sb")
        nc.sync.dma_start(out=b4sb[0:1, 0:1], in_=b4s[:, :])
        zconst = statics.tile([128, NT], BF16, tag="zconst")
        nc.vector.memset(zconst[:, :], 0.0)

        # L4 accumulator: one bank; nt0 sums at partition 0, nt1 at 32.
        acc = psacc.tile([128, NT], F32, tag="acc")
        # dummy matmul opens the accumulation group: start=True clears
        # has_written for the bank and writes zeros to partitions 0..32, so
        # every real L4 matmul can run start=False (overwrite-then-accum).
        nc.tensor.matmul(
            acc[0:33, :], zconst[0:1, 0:33], zconst[0:1, :],
            start=True, stop=False, skip_group_check=True,
        )

        # ---- HAM warmup: ~5us of dense, 4-way-overlapped matmuls ----
        wa = psl1.tile([128, b_core], F32, tag="zl1")
        wb = psl1.tile([128, b_core], F32, tag="zl1")
        wc = psl2.tile([128, b_core], F32, tag="zl2")
        wd = psl3.tile([128, NT], F32, tag="zl3")
        wtiles = [wa[:, 0:NT], wa[:, NT:], wb[:, 0:NT], wb[:, NT:],
                  wc[:, 0:NT], wc[:, NT:], wd[:, :]]
        for wi in range(28):
            t = wtiles[wi % 7]
            rp = 32 * (wi % 4)
            nc.tensor.matmul(
                t,
                zconst[rp : rp + 1, 0:128],
                zconst[rp : rp + 1, :],
                tile_position=(rp, 0),
                start=(wi < 7),
                stop=(wi >= 21),
            )

        h2_prev = None
        for p in range(n_pairs):
            ro = 64 * (p % 2)  # row-position base: {0,32} or {64,96}
            q = p // 2

            # ---- stream inputs/weights for this pair ----
            # xst rows carry [x-rows | W1 columns] for the K=5 split matmul
            xst = xpool.tile([128, b_core + 128], BF16, tag="xst")
            nc.sync.dma_start(out=xst[ro : ro + 5, :], in_=xti[p, 0])
            nc.sync.dma_start(out=xst[ro + 32 : ro + 37, :], in_=xti[p, 1])
            w2st = w2pool.tile([128, 128], BF16, tag="w2st")
            nc.gpsimd.dma_start(out=w2st[:, :], in_=w2p[p])
            if p % 2 == 0:
                w3st = w3pool.tile([128, 128], BF16, tag="w3st")
                nc.gpsimd.dma_start(out=w3st[:, :], in_=w3p[q])

            # ---- L1: z1 via K=5 split outer products ----
            zl1a = psl1.tile([128, b_core], F32, tag="zl1")
            zl1b = psl1.tile([128, b_core], F32, tag="zl1")
            for nt in range(2):
                s = slice(nt * NT, (nt + 1) * NT)
                nc.tensor.matmul(
                    zl1a[:, s],
                    xst[ro : ro + 5, b_core : b_core + 128],
                    xst[ro : ro + 5, s],
                    tile_position=(ro, 0),
                )
                nc.tensor.matmul(
                    zl1b[:, s],
                    xst[ro + 32 : ro + 37, b_core : b_core + 128],
                    xst[ro + 32 : ro + 37, s],
                    tile_position=(ro + 32, 0),
                )

            # ---- L1 evacuation (ACT): h1 = relu(z1), PSUM -> SBUF bf16 ----
            h1 = h1pool.tile([128, 2 * b_core], BF16, tag="h1")
            nc.scalar.activation(out=h1[:, 0:b_core], in_=zl1a[:, :], func=AF.Relu)
            if p % 4 == 3:
                nc.vector.tensor_scalar(
                    out=h1[:, b_core : 2 * b_core],
                    in0=zl1b[:, :],
                    scalar1=0.0,
                    scalar2=None,
                    op0=ALU.max,
                )
            else:
                nc.scalar.activation(
                    out=h1[:, b_core : 2 * b_core], in_=zl1b[:, :], func=AF.Relu
                )

            # ---- L2: z2 = W2^T h1 (K=128, M=64, two features col-packed) ----
            zl2 = psl2.tile([128, b_core], F32, tag="zl2")
            for nt in range(2):
                s = slice(nt * NT, (nt + 1) * NT)
                nc.tensor.matmul(
                    zl2[0:64, s],
                    w2st[:, 0:64],
                    h1[:, nt * NT : (nt + 1) * NT],
                    tile_position=(0, 0),
                )
                nc.tensor.matmul(
                    zl2[64:128, s],
                    w2st[:, 64:128],
                    h1[:, b_core + nt * NT : b_core + (nt + 1) * NT],
                    tile_position=(0, 64),
                )

            # ---- L2 evacuation (DVE): h2 = relu(z2 + b2) ----
            h2 = h2pool.tile([128, b_core], BF16, tag="h2")
            nc.vector.tensor_scalar(
                out=h2[:, :],
                in0=zl2[:, :],
                scalar1=b2s[:, p : p + 1],
                scalar2=0.0,
                op0=ALU.add,
                op1=ALU.max,
            )

            if p % 2 == 0:
                h2_prev = h2
                continue

            # ---- L3 (per quad): K=64, M=32, 4 features row+col packed ----
            h2a, h2b = h2_prev, h2
            h3 = h3pool.tile([128, b_core], BF16, tag="h3")
            for nt in range(2):
                s = slice(nt * NT, (nt + 1) * NT)
                zl3 = psl3.tile([128, NT], F32, tag="zl3")
                nc.tensor.matmul(
                    zl3[0:64, :], w3st[:, 0:64], h2a[:, s],
                    tile_position=(0, 0),
                )
                nc.tensor.matmul(
                    zl3[64:128, :], w3st[:, 64:128], h2b[:, s],
                    tile_position=(0, 64),
                )
                # ---- L3 evacuation (DVE): h3 = relu(z3 + b3) ----
                nc.vector.tensor_scalar(
                    out=h3[:, s],
                    in0=zl3[:, :],
                    scalar1=b3s[:, q : q + 1],
                    scalar2=0.0,
                    op0=ALU.add,
                    op1=ALU.max,
                )

            # ---- L4: y += W4^T h3 (K=128, M=1); nt0 -> partition 0,
            # nt1 -> partition 32 (column position 32), same bank ----
            nc.tensor.matmul(
                acc[0:1, :],
                w4s[:, q : q + 1],
                h3[:, 0:NT],
                tile_position=(0, 0),
                start=False,
                stop=False,
                skip_group_check=True,
            )
            nc.tensor.matmul(
                acc[32:33, :],
                w4s[:, q : q + 1],
                h3[:, NT : 2 * NT],
                tile_position=(0, 32),
                start=False,
                stop=(q == n_quads - 1),
                skip_group_check=True,
            )

        # ---- final: out[b] = acc + (sum(b4) + bias) ----
        outsb = finpool.tile([128, b_core], F32, tag="outsb")
        nc.vector.tensor_scalar(
            out=outsb[0:1, 0:NT],
            in0=acc[0:1, :],
            scalar1=b4sb[0:1, 0:1],
            scalar2=None,
            op0=ALU.add,
        )
        nc.vector.tensor_scalar(
            out=outsb[32:33, NT : 2 * NT],
            in0=acc[32:33, :],
            scalar1=b4sb[0:1, 0:1],
            scalar2=None,
            op0=ALU.add,
        )
        nc.sync.dma_start(out=out[0:1, 0:NT], in_=outsb[0:1, 0:NT])
        nc.sync.dma_start(out=out[0:1, NT : 2 * NT], in_=outsb[32:33, NT : 2 * NT])

    nc.compile()
    return nc


def _split_hi_lo(a):
    hi = a.astype(BF)
    lo = (a - hi.astype(np.float32)).astype(BF)
    return hi, lo


def pack_shared(W1, b1, W2, b2, W3, b3, W4, b4, bias, n_pairs):
    """Host-side packing of weights into the layouts the kernel streams."""
    n_quads = n_pairs // 2
    f4 = np.float32

    # L1 lhsT rows: [W1h; W1h; W1l; b1h; b1l] per feature
    w1h, w1l = _split_hi_lo(W1)
    b1h, b1l = _split_hi_lo(b1)
    w1b = np.empty((n_pairs, 2, 5, 128), BF)
    for s in range(2):
        w1b[:, s, 0, :] = w1h[s::2][:n_pairs]
        w1b[:, s, 1, :] = w1h[s::2][:n_pairs]
        w1b[:, s, 2, :] = w1l[s::2][:n_pairs]
        w1b[:, s, 3, :] = b1h[s::2][:n_pairs]
        w1b[:, s, 4, :] = b1l[s::2][:n_pairs]

    w2p = np.empty((n_pairs, 128, 128), BF)
    w2p[:, :, 0:64] = W2[0 : 2 * n_pairs : 2]
    w2p[:, :, 64:128] = W2[1 : 2 * n_pairs : 2]

    # block-diag over the h2 pair tiles: cols 0:63 <- (W3a, W3b),
    # cols 64:127 <- (W3c, W3d)
    w3p = np.zeros((n_quads, 128, 128), BF)
    w3p[:, 0:64, 0:32] = W3[0 : 4 * n_quads : 4]
    w3p[:, 64:128, 32:64] = W3[1 : 4 * n_quads : 4]
    w3p[:, 0:64, 64:96] = W3[2 : 4 * n_quads : 4]
    w3p[:, 64:128, 96:128] = W3[3 : 4 * n_quads : 4]

    w4f = W4[:, :, 0]  # [F, 32]
    w4p = np.empty((128, n_quads), BF)
    b3t = np.empty((128, n_quads), f4)
    for i in range(4):
        w4p[32 * i : 32 * (i + 1), :] = w4f[i : 4 * n_quads : 4].T
        b3t[32 * i : 32 * (i + 1), :] = b3[i : 4 * n_quads : 4].T

    b2t = np.empty((128, n_pairs), f4)
    b2t[0:64, :] = b2[0 : 2 * n_pairs : 2].T
    b2t[64:128, :] = b2[1 : 2 * n_pairs : 2].T

    b4v = np.array([[np.sum(b4) + float(bias[0])]], f4)
    return {
        "_w1b": w1b,
        "w2p": w2p,
        "w3p": w3p,
        "w4p": w4p,
        "b2p": b2t,
        "b3p": b3t,
        "b4s": b4v,
    }


def pack_x(x_core, n_pairs, w1b):
    """Per-core x staging rows: [xh; xl; xh; 1; 1 | W1/b1 cols] per slot."""
    b = x_core.shape[0]
    xT = np.ascontiguousarray(x_core.T.astype(np.float32))  # [F, B]
    xh, xl = _split_hi_lo(xT)
    xti = np.empty((n_pairs, 2, 5, b + 128), BF)
    for s in range(2):
        xti[:, s, 0, 0:b] = xh[s::2][:n_pairs]
        xti[:, s, 1, 0:b] = xl[s::2][:n_pairs]
        xti[:, s, 2, 0:b] = xh[s::2][:n_pairs]
    xti[:, :, 3:5, 0:b] = BF(1.0)
    xti[:, :, :, b:] = w1b
    return xti


_PROGRAM_CACHE = {}


def _get_program(n_pairs):
    if n_pairs not in _PROGRAM_CACHE:
        _PROGRAM_CACHE[n_pairs] = build_program(n_pairs=n_pairs)
    return _PROGRAM_CACHE[n_pairs]


def kernel(x, W1, b1, W2, b2, W3, b3, W4, b4, bias, _trace=False):
    x = np.asarray(x, np.float32)
    args = [np.asarray(a, np.float32) for a in (W1, b1, W2, b2, W3, b3, W4, b4, bias)]
    W1, b1, W2, b2, W3, b3, W4, b4, bias = args

    B, F = x.shape
    n_pairs = F // 2
    bc = B // N_CORES
    assert bc == B_CORE, f"expected {B_CORE} rows/core, got {bc}"

    shared = pack_shared(W1, b1, W2, b2, W3, b3, W4, b4, bias, n_pairs)
    w1b = shared.pop("_w1b")
    in_maps = []
    for c in range(N_CORES):
        m = dict(shared)
        m["xti"] = pack_x(x[c * bc : (c + 1) * bc], n_pairs, w1b)
        in_maps.append(m)

    nc = _get_program(n_pairs)
    res = run_bass_kernel_spmd(
        nc, in_maps, core_ids=list(range(N_CORES)), trace=_trace
    )
    out = np.concatenate(
        [res.results[c]["out"].reshape(bc, 1) for c in range(N_CORES)], axis=0
    )
    if _trace:
        kernel.last_results = res
    return out.astype(np.float32)


# revision 18
# speedup vs baseline: 1.6105x; 1.0002x over previous
"""Trainium2 Bass kernel for a Neural Additive Model (dense per-feature MLPs).

Math (per batch row b, feature f):
    h1 = relu(x[b,f] * W1[f] + b1[f])          # [128]
    h2 = relu(W2[f]^T h1 + b2[f])              # [64]
    h3 = relu(W3[f]^T h2 + b3[f])              # [32]
    y  = sum_f (W4[f]^T h3 + b4[f]) + bias     # scalar
Output: [B, 1].

Distribution: data-parallel over batch across 8 NeuronCores (B=8192 -> 1024
per core), weights replicated; no collectives, host concatenates outputs.

Per-core dataflow ([hidden-on-partition, batch-on-free] layout), v3:
  L1: PE outer products, K=5 bf16 hi/lo split (exact to ~1e-5):
      z1 = W1h(x)xh + W1h(x)xl + W1l(x)xh + b1h(x)1 + b1l(x)1.
      Features pair-pipelined, alternating row tile positions.
  L1/L2/L3 PSUM evacuation: relu (+bias for L2/L3) fused into the single
      PSUM->SBUF pass; ACT takes h1, DVE takes h2/h3.
  L2: bf16 K=128,M=64 matmuls, 2 features packed via column tiling.
  L3: bf16 K=64,M=32 matmuls, 4 features packed via row+column tiling.
  L4: bf16 K=128,M=1 matmuls accumulating all features into one PSUM bank
      (nt0 at partition 0, nt1 at partition 32 via column position 32);
      a zero dummy matmul opens the accumulation group.
"""

from contextlib import ExitStack

import numpy as np
import ml_dtypes

import concourse.bass as bass
import concourse.tile as tile
from concourse import bacc, mybir
from concourse.bass_utils import run_bass_kernel_spmd

F32 = mybir.dt.float32
BF16 = mybir.dt.float16
AF = mybir.ActivationFunctionType
ALU = mybir.AluOpType
BF = np.float16

N_CORES = 8
B_CORE = 1024  # batch rows per core
NT = 512  # moving-dim tile (one fp32 PSUM bank)


def build_program(n_pairs=128, b_core=B_CORE):
    """Build the per-core Bass program (SPMD: same program on all cores)."""
    assert n_pairs % 2 == 0
    n_quads = n_pairs // 2

    nc = bacc.Bacc("TRN2", target_bir_lowering=False, debug=False)

    xti = nc.dram_tensor("xti", [n_pairs, 2, 5, b_core + 128], BF16, kind="ExternalInput")
    w2p = nc.dram_tensor("w2p", [n_pairs, 128, 128], BF16, kind="ExternalInput")
    w3p = nc.dram_tensor("w3p", [n_quads, 128, 128], BF16, kind="ExternalInput")
    w4p = nc.dram_tensor("w4p", [128, n_quads], BF16, kind="ExternalInput")
    b2p = nc.dram_tensor("b2p", [128, n_pairs], F32, kind="ExternalInput")
    b3p = nc.dram_tensor("b3p", [128, n_quads], F32, kind="ExternalInput")
    b4s = nc.dram_tensor("b4s", [1, 1], F32, kind="ExternalInput")
    out = nc.dram_tensor("out", [1, b_core], F32, kind="ExternalOutput")

    with tile.TileContext(nc) as tc, ExitStack() as ctx:
        statics = ctx.enter_context(tc.tile_pool(name="statics", bufs=1))
        xpool = ctx.enter_context(tc.tile_pool(name="xpool", bufs=3))
        w2pool = ctx.enter_context(tc.tile_pool(name="w2pool", bufs=3))
        w3pool = ctx.enter_context(tc.tile_pool(name="w3pool", bufs=2))
        h1pool = ctx.enter_context(tc.tile_pool(name="h1pool", bufs=3))
        h2pool = ctx.enter_context(tc.tile_pool(name="h2pool", bufs=4))
        h3pool = ctx.enter_context(tc.tile_pool(name="h3pool", bufs=2))
        finpool = ctx.enter_context(tc.tile_pool(name="finpool", bufs=1))
        psl1 = ctx.enter_context(tc.tile_pool(name="psl1", bufs=2, space="PSUM"))
        psl2 = ctx.enter_context(tc.tile_pool(name="psl2", bufs=1, space="PSUM"))
        psl3 = ctx.enter_context(tc.tile_pool(name="psl3", bufs=1, space="PSUM"))
        psacc = ctx.enter_context(tc.tile_pool(name="psacc", bufs=1, space="PSUM"))

        # static staging
        b2s = statics.tile([128, n_pairs], F32, tag="b2s")
        nc.sync.dma_start(out=b2s[:, :], in_=b2p[:, :])
        b3s = statics.tile([128, n_quads], F32, tag="b3s")
        nc.sync.dma_start(out=b3s[:, :], in_=b3p[:, :])
        w4s = statics.tile([128, n_quads], BF16, tag="w4s")
        nc.sync.dma_start(out=w4s[:, :], in_=w4p[:, :])
        b4sb = statics.tile([128, 1], F32, tag="b4sb")
        nc.sync.dma_start(out=b4sb[0:1, 0:1], in_=b4s[:, :])
        zconst = statics.tile([128, NT], BF16, tag="zconst")
        nc.vector.memset(zconst[:, :], 0.0)

        # L4 accumulator: one bank; nt0 sums at partition 0, nt1 at 32.
        acc = psacc.tile([128, NT], F32, tag="acc")
        # dummy matmul opens the accumulation group: start=True clears
        # has_written for the bank and writes zeros to partitions 0..32, so
        # every real L4 matmul can run start=False (overwrite-then-accum).
        nc.tensor.matmul(
            acc[0:33, :], zconst[0:1, 0:33], zconst[0:1, :],
            start=True, stop=False, skip_group_check=True,
        )

        # ---- HAM warmup: ~10us of full-array matmuls (K=128, M=128) so
        # the PE activity monitor releases the clock gate (1.2 -> 2.4 GHz)
        wa = psl1.tile([128, b_core], F32, tag="zl1")
        for wi in range(16):
            nc.tensor.matmul(
                wa[:, 0:NT] if wi % 2 == 0 else wa[:, NT:],
                zconst[:, 0:128],
                zconst[:, :],
                start=(wi < 2),
                stop=(wi >= 14),
            )

        h2_prev = None
        for p in range(n_pairs):
            ro = 64 * (p % 2)  # row-position base: {0,32} or {64,96}
            q = p // 2

            # ---- stream inputs/weights for this pair ----
            # xst rows carry [x-rows | W1 columns] for the K=5 split matmul
            xst = xpool.tile([128, b_core + 128], BF16, tag="xst")
            nc.sync.dma_start(out=xst[ro : ro + 5, :], in_=xti[p, 0])
            nc.sync.dma_start(out=xst[ro + 32 : ro + 37, :], in_=xti[p, 1])
            w2st = w2pool.tile([128, 128], BF16, tag="w2st")
            nc.gpsimd.dma_start(out=w2st[:, :], in_=w2p[p])
            if p % 2 == 0:
                w3st = w3pool.tile([128, 128], BF16, tag="w3st")
                nc.gpsimd.dma_start(out=w3st[:, :], in_=w3p[q])

            # ---- L1: z1 via K=5 split outer products ----
            zl1a = psl1.tile([128, b_core], F32, tag="zl1")
            zl1b = psl1.tile([128, b_core], F32, tag="zl1")
            for nt in range(2):
                s = slice(nt * NT, (nt + 1) * NT)
                nc.tensor.matmul(
                    zl1a[:, s],
                    xst[ro : ro + 5, b_core : b_core + 128],
                    xst[ro : ro + 5, s],
                    tile_position=(ro, 0),
                )
                nc.tensor.matmul(
                    zl1b[:, s],
                    xst[ro + 32 : ro + 37, b_core : b_core + 128],
                    xst[ro + 32 : ro + 37, s],
                    tile_position=(ro + 32, 0),
                )

            # ---- L1 evacuation (ACT): h1 = relu(z1), PSUM -> SBUF bf16 ----
            h1 = h1pool.tile([128, 2 * b_core], BF16, tag="h1")
            nc.scalar.activation(out=h1[:, 0:b_core], in_=zl1a[:, :], func=AF.Relu)
            if p % 4 == 3:
                nc.vector.tensor_scalar(
                    out=h1[:, b_core : 2 * b_core],
                    in0=zl1b[:, :],
                    scalar1=0.0,
                    scalar2=None,
                    op0=ALU.max,
                )
            else:
                nc.scalar.activation(
                    out=h1[:, b_core : 2 * b_core], in_=zl1b[:, :], func=AF.Relu
                )

            # ---- L2: z2 = W2^T h1 (K=128, M=64, two features col-packed) ----
            zl2 = psl2.tile([128, b_core], F32, tag="zl2")
            for nt in range(2):
                s = slice(nt * NT, (nt + 1) * NT)
                nc.tensor.matmul(
                    zl2[0:64, s],
                    w2st[:, 0:64],
                    h1[:, nt * NT : (nt + 1) * NT],
                    tile_position=(0, 0),
                )
                nc.tensor.matmul(
                    zl2[64:128, s],
                    w2st[:, 64:128],
                    h1[:, b_core + nt * NT : b_core + (nt + 1) * NT],
                    tile_position=(0, 64),
                )

            # ---- L2 evacuation (DVE): h2 = relu(z2 + b2) ----
            h2 = h2pool.tile([128, b_core], BF16, tag="h2")
            nc.vector.tensor_scalar(
                out=h2[:, :],
                in0=zl2[:, :],
                scalar1=b2s[:, p : p + 1],
                scalar2=0.0,
                op0=ALU.add,
                op1=ALU.max,
            )

            if p % 2 == 0:
                h2_prev = h2
                continue

            # ---- L3 (per quad): K=64, M=32, 4 features row+col packed ----
            h2a, h2b = h2_prev, h2
            h3 = h3pool.tile([128, b_core], BF16, tag="h3")
            for nt in range(2):
                s = slice(nt * NT, (nt + 1) * NT)
                zl3 = psl3.tile([128, NT], F32, tag="zl3")
                nc.tensor.matmul(
                    zl3[0:64, :], w3st[:, 0:64], h2a[:, s],
                    tile_position=(0, 0),
                )
                nc.tensor.matmul(
                    zl3[64:128, :], w3st[:, 64:128], h2b[:, s],
                    tile_position=(0, 64),
                )
                # ---- L3 evacuation (DVE): h3 = relu(z3 + b3) ----
                nc.vector.tensor_scalar(
                    out=h3[:, s],
                    in0=zl3[:, :],
                    scalar1=b3s[:, q : q + 1],
                    scalar2=0.0,
                    op0=ALU.add,
                    op1=ALU.max,
                )

            # ---- L4: y += W4^T h3 (K=128, M=1); nt0 -> partition 0,
            # nt1 -> partition 32 (column position 32), same bank ----
            nc.tensor.matmul(
                acc[0:1, :],
                w4s[:, q : q + 1],
                h3[:, 0:NT],
                tile_position=(0, 0),
                start=False,
                stop=False,
                skip_group_check=True,
            )
            nc.tensor.matmul(
                acc[32:33, :],
                w4s[:, q : q + 1],
                h3[:, NT : 2 * NT],
                tile_position=(0, 32),
                start=False,
                stop=(q == n_quads - 1),
                skip_group_check=True,
            )

        # ---- final: out[b] = acc + (sum(b4) + bias) ----
        outsb = finpool.tile([128, b_core], F32, tag="outsb")
        nc.vector.tensor_scalar(
            out=outsb[0:1, 0:NT],
            in0=acc[0:1, :],
            scalar1=b4sb[0:1, 0:1],
            scalar2=None,
            op0=ALU.add,
        )
        nc.vector.tensor_scalar(
            out=outsb[32:33, NT : 2 * NT],
            in0=acc[32:33, :],
            scalar1=b4sb[0:1, 0:1],
            scalar2=None,
            op0=ALU.add,
        )
        nc.sync.dma_start(out=out[0:1, 0:NT], in_=outsb[0:1, 0:NT])
        nc.sync.dma_start(out=out[0:1, NT : 2 * NT], in_=outsb[32:33, NT : 2 * NT])

    nc.compile()
    return nc


def _split_hi_lo(a):
    hi = a.astype(BF)
    lo = (a - hi.astype(np.float32)).astype(BF)
    return hi, lo


def pack_shared(W1, b1, W2, b2, W3, b3, W4, b4, bias, n_pairs):
    """Host-side packing of weights into the layouts the kernel streams."""
    n_quads = n_pairs // 2
    f4 = np.float32

    # L1 lhsT rows: [W1h; W1h; W1l; b1h; b1l] per feature
    w1h, w1l = _split_hi_lo(W1)
    b1h, b1l = _split_hi_lo(b1)
    w1b = np.empty((n_pairs, 2, 5, 128), BF)
    for s in range(2):
        w1b[:, s, 0, :] = w1h[s::2][:n_pairs]
        w1b[:, s, 1, :] = w1h[s::2][:n_pairs]
        w1b[:, s, 2, :] = w1l[s::2][:n_pairs]
        w1b[:, s, 3, :] = b1h[s::2][:n_pairs]
        w1b[:, s, 4, :] = b1l[s::2][:n_pairs]

    w2p = np.empty((n_pairs, 128, 128), BF)
    w2p[:, :, 0:64] = W2[0 : 2 * n_pairs : 2]
    w2p[:, :, 64:128] = W2[1 : 2 * n_pairs : 2]

    # block-diag over the h2 pair tiles: cols 0:63 <- (W3a, W3b),
    # cols 64:127 <- (W3c, W3d)
    w3p = np.zeros((n_quads, 128, 128), BF)
    w3p[:, 0:64, 0:32] = W3[0 : 4 * n_quads : 4]
    w3p[:, 64:128, 32:64] = W3[1 : 4 * n_quads : 4]
    w3p[:, 0:64, 64:96] = W3[2 : 4 * n_quads : 4]
    w3p[:, 64:128, 96:128] = W3[3 : 4 * n_quads : 4]

    w4f = W4[:, :, 0]  # [F, 32]
    w4p = np.empty((128, n_quads), BF)
    b3t = np.empty((128, n_quads), f4)
    for i in range(4):
        w4p[32 * i : 32 * (i + 1), :] = w4f[i : 4 * n_quads : 4].T
        b3t[32 * i : 32 * (i + 1), :] = b3[i : 4 * n_quads : 4].T

    b2t = np.empty((128, n_pairs), f4)
    b2t[0:64, :] = b2[0 : 2 * n_pairs : 2].T
    b2t[64:128, :] = b2[1 : 2 * n_pairs : 2].T

    b4v = np.array([[np.sum(b4) + float(bias[0])]], f4)
    return {
        "_w1b": w1b,
        "w2p": w2p,
        "w3p": w3p,
        "w4p": w4p,
        "b2p": b2t,
        "b3p": b3t,
        "b4s": b4v,
    }


def pack_x(x_core, n_pairs, w1b):
    """Per-core x staging rows: [xh; xl; xh; 1; 1 | W1/b1 cols] per slot."""
    b = x_core.shape[0]
    xT = np.ascontiguousarray(x_core.T.astype(np.float32))  # [F, B]
    xh, xl = _split_hi_lo(xT)
    xti = np.empty((n_pairs, 2, 5, b + 128), BF)
    for s in range(2):
        xti[:, s, 0, 0:b] = xh[s::2][:n_pairs]
        xti[:, s, 1, 0:b] = xl[s::2][:n_pairs]
        xti[:, s, 2, 0:b] = xh[s::2][:n_pairs]
    xti[:, :, 3:5, 0:b] = BF(1.0)
    xti[:, :, :, b:] = w1b
    return xti


_PROGRAM_CACHE = {}


def _get_program(n_pairs):
    if n_pairs not in _PROGRAM_CACHE:
        _PROGRAM_CACHE[n_pairs] = build_program(n_pairs=n_pairs)
    return _PROGRAM_CACHE[n_pairs]


def kernel(x, W1, b1, W2, b2, W3, b3, W4, b4, bias, _trace=False):
    x = np.asarray(x, np.float32)
    args = [np.asarray(a, np.float32) for a in (W1, b1, W2, b2, W3, b3, W4, b4, bias)]
    W1, b1, W2, b2, W3, b3, W4, b4, bias = args

    B, F = x.shape
    n_pairs = F // 2
    bc = B // N_CORES
    assert bc == B_CORE, f"expected {B_CORE} rows/core, got {bc}"

    shared = pack_shared(W1, b1, W2, b2, W3, b3, W4, b4, bias, n_pairs)
    w1b = shared.pop("_w1b")
    in_maps = []
    for c in range(N_CORES):
        m = dict(shared)
        m["xti"] = pack_x(x[c * bc : (c + 1) * bc], n_pairs, w1b)
        in_maps.append(m)

    nc = _get_program(n_pairs)
    res = run_bass_kernel_spmd(
        nc, in_maps, core_ids=list(range(N_CORES)), trace=_trace
    )
    out = np.concatenate(
        [res.results[c]["out"].reshape(bc, 1) for c in range(N_CORES)], axis=0
    )
    if _trace:
        kernel.last_results = res
    return out.astype(np.float32)
